# revision 17
# baseline (speedup 1.0000x reference)
"""Trainium2 Bass kernel for nn_Attention_69861938037658.

Computation per batch b (B=4096, S=200, H=128):
    proj  = X_b @ W1.T + (l_b @ W2.T)          # [S,H]
    hid   = tanh(proj)
    sc    = hid @ W3_w.T                       # [S]
    sc    = where(mask, -1e9, sc)
    attn  = softmax(sc)
    out_b = attn @ X_b                         # [H]

Sharding: pure data parallel, 512 batches per core on 8 cores.

Per-core structure (BB=64 batches per block, 8 blocks):
  - X loaded as [s, b, h]; per 2-batch group, PE-transpose to X^T columns,
    W1 matmul, tanh(+W2 l bias) on ACT.
  - Scores for all 32 groups of a block accumulate into one [32, 400] PSUM
    tile via "w3sel" stationaries ([128, 32] with w3 in column g, zeros
    elsewhere) - no per-row scatter DMAs.
  - Blockwide masked softmax on the [32, 400] two-segment layout.
  - Weighted sum via per-batch stationary-X matmuls into spaced PSUM cols.
"""

import sys
import numpy as np

if "/opt/trn_rl_repo" not in sys.path:
    sys.path.insert(0, "/opt/trn_rl_repo")

B, S, H = 4096, 200, 128
NCORES = 8
BC = B // NCORES          # 512 batches per core
BB = 64                   # batches per block
NBLK = BC // BB           # 8 blocks
NEG = -1.0e9

_NC_CACHE = {}
_RUN_CACHE = {}


def _build(nblk=NBLK, bb=BB, reps=1):
    import concourse.bacc as bacc
    import concourse.tile as tile
    from concourse import mybir
    from concourse.masks import make_identity
    from contextlib import ExitStack

    f32 = mybir.dt.float32
    f32r = mybir.dt.float32r
    u8 = mybir.dt.uint8
    Tanh = mybir.ActivationFunctionType.Tanh
    Exp = mybir.ActivationFunctionType.Exp

    bcp = nblk * bb           # batches this core processes
    gb = bb // 2              # 2-batch groups per block

    nc = bacc.Bacc("TRN2", target_bir_lowering=False, debug=False)

    x = nc.dram_tensor("x", [bcp, S, H], f32r, kind="ExternalInput")
    l = nc.dram_tensor("l", [bcp, H], f32, kind="ExternalInput")
    m = nc.dram_tensor("m", [bcp, S], u8, kind="ExternalInput")
    w1 = nc.dram_tensor("w1", [H, H], f32, kind="ExternalInput")
    w2 = nc.dram_tensor("w2", [H, H], f32, kind="ExternalInput")
    w3 = nc.dram_tensor("w3", [1, H], f32, kind="ExternalInput")
    out = nc.dram_tensor("out", [bcp, H], f32, kind="ExternalOutput")

    with tile.TileContext(nc) as tc, ExitStack() as ctx:
        singles = ctx.enter_context(tc.tile_pool(name="singles", bufs=1))
        xa_p = ctx.enter_context(tc.tile_pool(name="xa", bufs=2))
        xb_p = ctx.enter_context(tc.tile_pool(name="xb", bufs=2))
        xt_p = ctx.enter_context(tc.tile_pool(name="xt", bufs=4))
        hid_p = ctx.enter_context(tc.tile_pool(name="hid", bufs=4))
        sm_p = ctx.enter_context(tc.tile_pool(name="sm", bufs=2))
        small_p = ctx.enter_context(tc.tile_pool(name="small", bufs=3))
        o_p = ctx.enter_context(tc.tile_pool(name="o", bufs=2))
        xtps_p = ctx.enter_context(tc.tile_pool(name="xtps", bufs=2, space="PSUM"))
        pjps_p = ctx.enter_context(tc.tile_pool(name="pjps", bufs=2, space="PSUM"))
        scps_p = ctx.enter_context(tc.tile_pool(name="scps", bufs=2, space="PSUM"))
        fob_p = ctx.enter_context(tc.tile_pool(name="fob", bufs=2, space="PSUM"))

        # ---- constants / weights ----
        ident = singles.tile([128, 128], f32)
        make_identity(nc, ident)
        negt = singles.tile([gb, 2 * S], f32)
        nc.vector.memset(negt, NEG)

        w1sb = singles.tile([H, H], f32)
        w2sb = singles.tile([H, H], f32)
        w3sb = singles.tile([1, H], f32)
        nc.sync.dma_start(out=w1sb, in_=w1[:, :])
        nc.sync.dma_start(out=w2sb, in_=w2[:, :])
        nc.sync.dma_start(out=w3sb, in_=w3[:, :])

        identr = singles.tile([128, 128], f32r)
        nc.vector.tensor_copy(identr, ident)
        wfob = fob_p.tile([128, 512], f32, tag="fob")
        w1T = singles.tile([H, H], f32r)
        nc.tensor.transpose(wfob[:, 0:H], w1sb, ident)
        nc.vector.tensor_copy(w1T, wfob[:, 0:H])
        wfob2 = fob_p.tile([128, 512], f32, tag="fob")
        w2T = singles.tile([H, H], f32r)
        nc.tensor.transpose(wfob2[:, 0:H], w2sb, ident)
        nc.vector.tensor_copy(w2T, wfob2[:, 0:H])

        # w3sel[:, g, :] is a [H, gb] stationary: w3 in column g, else 0.
        nc.tensor.transpose(wfob2[:, 256:257], w3sb, ident[0:1, 0:1])
        w3self = singles.tile([H, gb, gb], f32)
        nc.vector.memset(w3self, 0.0)
        for g in range(gb):
            nc.vector.tensor_copy(w3self[:, g, g : g + 1], wfob2[:, 256:257])
        w3sel = singles.tile([H, gb, gb], f32r)
        nc.vector.tensor_copy(w3sel, w3self)

        for rep in range(reps):
            for blk in range(nblk):
                b0 = blk * bb

                # one psum scratch tile per block:
                #   cols 0:128   fo   (weighted-sum accum, spacing 2)
                #   cols 128:256 atps (attn transposes, 4 x gb)
                #   cols 256:320 ltps; 320:448 plnat (onps reuses 320:448
                #   at block end)
                fob = fob_p.tile([128, 512], f32, tag="fob")
                fo = fob[:, 0 : 2 * bb]
                atps = fob[:, 2 * bb : 2 * bb + 4 * gb].rearrange(
                    "p (a g) -> p a g", a=4)

                # ---- proj_last for this block: plt[h, b] = W2 @ L_blk^T ----
                lsb = small_p.tile([bb, H], f32, tag="lsb")
                nc.sync.dma_start(out=lsb, in_=l[b0 : b0 + bb, :])
                nc.tensor.transpose(fob[:, 256 : 256 + bb], lsb, ident[0:bb, 0:bb])
                lt = small_p.tile([H, bb], f32r, tag="lt")
                nc.vector.tensor_copy(lt, fob[:, 256 : 256 + bb])
                nc.tensor.matmul(fob[:, 320 : 320 + bb], w2T, lt, start=True, stop=True)
                plt = small_p.tile([H, bb], f32r, tag="plt")
                nc.vector.tensor_copy(plt, fob[:, 320 : 320 + bb])

                # mask in [g, (pair s)] layout: row g = batches 2g, 2g+1
                mskt = small_p.tile([gb, 2 * S], u8, tag="msk")
                nc.sync.dma_start(
                    out=mskt,
                    in_=m[b0 : b0 + bb, :].rearrange("(g t) s -> g (t s)", t=2),
                )

                # ---- X loads: [s, b, h] ----
                xa = xa_p.tile([128, bb, H], f32r)
                xb = xb_p.tile([72, bb, H], f32r)
                nc.sync.dma_start(
                    out=xa, in_=x[b0 : b0 + bb, 0:128, :].rearrange("b s h -> s b h"))
                nc.sync.dma_start(
                    out=xb, in_=x[b0 : b0 + bb, 128:200, :].rearrange("b s h -> s b h"))

                # ---- per 2-batch group: transpose -> proj -> tanh -> scores
                # software-pipelined: w1 lags transposes by 1 group, w3sel
                # lags by 2, so PE never waits on DVE copies / ACT tanh.
                scps = scps_p.tile([gb, 2 * S], f32)
                xts = {}
                hids = {}

                def stage_transpose(g):
                    i0, i1 = 2 * g, 2 * g + 1
                    xtps = xtps_p.tile([128, 400], f32r)
                    nc.tensor.transpose(xtps[:, 0:128], xa[:, i0, :], identr)
                    nc.tensor.transpose(xtps[:, 128:200], xb[:, i0, :], identr[0:72, 0:72])
                    nc.tensor.transpose(xtps[:, 200:328], xa[:, i1, :], identr)
                    nc.tensor.transpose(xtps[:, 328:400], xb[:, i1, :], identr[0:72, 0:72])
                    xt = xt_p.tile([128, 400], f32r)
                    nc.vector.tensor_copy(xt, xtps)
                    xts[g] = xt

                def stage_proj(g):
                    i0 = 2 * g
                    pjps = pjps_p.tile([128, 400], f32)
                    nc.tensor.matmul(pjps, w1T, xts.pop(g), start=True, stop=False)
                    # + l @ W2^T per batch: identity stationary, plt columns
                    # broadcast along s as the moving operand
                    pl_bcast = plt[:, i0 : i0 + 2][:, :, None].broadcast_to([H, 2, S])
                    nc.tensor.matmul(pjps.rearrange("p (t s) -> p t s", t=2),
                                     identr, pl_bcast, start=False, stop=True)
                    hid = hid_p.tile([128, 400], f32r)
                    nc.scalar.activation(hid, pjps, Tanh)
                    hids[g] = hid

                def stage_score(g):
                    nc.tensor.matmul(scps, w3sel[:, g, :], hids.pop(g),
                                     start=(g == 0), stop=(g == gb - 1),
                                     skip_group_check=True)

                for g in range(gb + 2):
                    if g < gb:
                        stage_transpose(g)
                    if 1 <= g < gb + 1:
                        stage_proj(g - 1)
                    if g >= 2:
                        stage_score(g - 2)

                # ---- masked softmax over S, rows = 2-batch groups ----
                # scores are bounded by ||w3||_1 (~10), so exp without
                # max-subtraction is safe in f32; masked entries underflow
                # to exactly 0.
                sc = sm_p.tile([gb, 2 * S], f32, tag="sc")
                nc.vector.tensor_copy(sc, scps)
                nc.vector.copy_predicated(sc, mskt, negt)
                pb = sm_p.tile([gb, 2 * S], f32, tag="pb")
                zE = small_p.tile([gb, 1], f32, tag="zE")
                zO = small_p.tile([gb, 1], f32, tag="zO")
                nc.scalar.activation(pb, sc, Exp)
                nc.vector.tensor_reduce(zE, pb[:, 0:200], mybir.AxisListType.X,
                                        mybir.AluOpType.add)
                nc.vector.tensor_reduce(zO, pb[:, 200:400], mybir.AxisListType.X,
                                        mybir.AluOpType.add)
                rzE = small_p.tile([gb, 1], f32, tag="rzE")
                rzO = small_p.tile([gb, 1], f32, tag="rzO")
                nc.vector.reciprocal(rzE, zE)
                nc.vector.reciprocal(rzO, zO)
                attn = sm_p.tile([gb, 2 * S], f32, tag="attn")
                nc.vector.tensor_scalar_mul(attn[:, 0:200], pb[:, 0:200], rzE)
                nc.vector.tensor_scalar_mul(attn[:, 200:400], pb[:, 200:400], rzO)

                # ---- transpose attn into per-batch columns ----
                # segments: 0 = even batches s0:128, 1 = even s128:200,
                #           2 = odd  batches s0:128, 3 = odd  s128:200
                nc.tensor.transpose(atps[:, 0, :], attn[:, 0:128], ident[0:gb, 0:gb])
                nc.tensor.transpose(atps[0:72, 1, :], attn[:, 128:200], ident[0:gb, 0:gb])
                nc.tensor.transpose(atps[:, 2, :], attn[:, 200:328], ident[0:gb, 0:gb])
                nc.tensor.transpose(atps[0:72, 3, :], attn[:, 328:400], ident[0:gb, 0:gb])
                attT = small_p.tile([128, 4, gb], f32, tag="attT")
                nc.vector.tensor_copy(attT[:, 0:4:2, :], atps[:, 0:4:2, :])
                nc.vector.tensor_copy(attT[0:72, 1:4:2, :], atps[0:72, 1:4:2, :])

                # ---- weighted sum: fo[h, 2*i] = sum_s attn[s,i] * X[s,i,h]
                # (spacing 2 keeps each accumulation column on its own 8B
                # PSUM cacheline)
                for i in range(bb):
                    g, odd = divmod(i, 2)
                    ca = attT[:, 2 * odd, g : g + 1]
                    cb = attT[0:72, 2 * odd + 1, g : g + 1]
                    nc.tensor.matmul(fo[:, 2 * i : 2 * i + 1],
                                     xa[:, i, :].bitcast(f32), ca,
                                     start=True, stop=False)
                    nc.tensor.matmul(fo[:, 2 * i : 2 * i + 1],
                                     xb[:, i, :].bitcast(f32), cb,
                                     start=False, stop=True)

                outT = o_p.tile([128, bb], f32, tag="outT")
                nc.vector.tensor_copy(outT, fo[:, 0 : 2 * bb : 2])
                nc.tensor.transpose(fob[0:bb, 320:448], outT, ident)
                onat = o_p.tile([bb, H], f32, tag="onat")
                nc.vector.tensor_copy(onat, fob[0:bb, 320:448])
                nc.sync.dma_start(out=out[b0 : b0 + bb, :], in_=onat)

    nc.finalize()
    return nc


def _get_nc(nblk=NBLK, bb=BB, reps=1):
    key = (nblk, bb, reps)
    if key not in _NC_CACHE:
        _NC_CACHE[key] = _build(nblk, bb, reps)
    return _NC_CACHE[key]


def _get_runner(nc, n_cores=NCORES):
    """Compile nc into a cached sharded-jit callable over n_cores devices.

    Returns (fn, in_names, out_names) where fn takes GLOBAL input arrays
    (axis 0 = n_cores * per-core) in in_names order and returns global
    output arrays.
    """
    key = id(nc)
    if key in _RUN_CACHE:
        return _RUN_CACHE[key]
    import jax
    from jax.sharding import Mesh, PartitionSpec
    try:
        from jax.experimental.shard_map import shard_map
    except Exception:
        from jax.shard_map import shard_map
    from concourse import mybir
    from concourse import bass2jax

    bass2jax.install_neuronx_cc_hook()

    partition_name = (nc.partition_id_tensor.name
                      if nc.partition_id_tensor else None)
    in_names, out_names, out_avals, zero_shapes = [], [], [], []
    for alloc in nc.m.functions[0].allocations:
        if not isinstance(alloc, mybir.MemoryLocationSet):
            continue
        name = alloc.memorylocations[0].name
        if alloc.kind == "ExternalInput":
            if name != partition_name:
                in_names.append(name)
        elif alloc.kind == "ExternalOutput":
            out_names.append(name)
            shape = tuple(alloc.tensor_shape)
            dtype = mybir.dt.np(alloc.dtype)
            out_avals.append(jax.core.ShapedArray(shape, dtype))
            zero_shapes.append((shape, dtype))
    n_params = len(in_names)
    n_outs = len(out_names)
    all_names = in_names + out_names
    if partition_name is not None:
        all_names = all_names + [partition_name]
    donate = tuple(range(n_params, n_params + n_outs))

    def _body(*args):
        operands = list(args)
        if partition_name is not None:
            operands.append(bass2jax.partition_id_tensor())
        outs = bass2jax._bass_exec_p.bind(
            *operands,
            out_avals=tuple(out_avals),
            in_names=tuple(all_names),
            out_names=tuple(out_names),
            lowering_input_output_aliases=(),
            sim_require_finite=True,
            sim_require_nnan=True,
            nc=nc,
        )
        return tuple(outs)

    devices = jax.devices()[:n_cores]
    assert len(devices) == n_cores, f"need {n_cores} devices, have {len(jax.devices())}"
    mesh = Mesh(np.asarray(devices), ("core",))
    in_specs = (PartitionSpec("core"),) * (n_params + n_outs)
    out_specs = (PartitionSpec("core"),) * n_outs
    fn = jax.jit(
        shard_map(_body, mesh=mesh, in_specs=in_specs, out_specs=out_specs,
                  check_rep=False),
        donate_argnums=donate,
        keep_unused=True,
    )
    entry = (fn, in_names, out_names, zero_shapes)
    _RUN_CACHE[key] = entry
    return entry


def _global_inputs(all_memory, last_memory, mask, W1, W2, W3_w):
    """Name -> global array (axis 0 shards across cores with zero copies
    for the big tensors)."""
    return {
        "x": np.ascontiguousarray(all_memory, dtype=np.float32),
        "l": np.ascontiguousarray(last_memory.reshape(B, H), dtype=np.float32),
        "m": np.ascontiguousarray(mask).view(np.uint8),
        "w1": np.tile(np.ascontiguousarray(W1, dtype=np.float32), (NCORES, 1)),
        "w2": np.tile(np.ascontiguousarray(W2, dtype=np.float32), (NCORES, 1)),
        "w3": np.tile(np.ascontiguousarray(W3_w, dtype=np.float32), (NCORES, 1)),
    }


def run_global(in_map, nc=None, reps=1):
    """Run the kernel on global inputs; returns dict of global outputs."""
    if nc is None:
        nc = _get_nc(reps=reps)
    fn, in_names, out_names, zero_shapes = _get_runner(nc)
    args = [in_map[name] for name in in_names]
    zeros = [np.zeros((NCORES * s[0], *s[1:]), d) for (s, d) in zero_shapes]
    outs = fn(*args, *zeros)
    return {name: np.asarray(outs[i]) for i, name in enumerate(out_names)}


def kernel(all_memory, last_memory, mask, W1, W2, W3_w, W3_b):
    # W3_b shifts every score equally; softmax is shift-invariant, so it
    # cancels (and it is zeros in setup_inputs).
    in_map = _global_inputs(all_memory, last_memory, mask, W1, W2, W3_w)
    outs = run_global(in_map)
    return outs["out"].astype(np.float32)


# revision 19
# speedup vs baseline: 1.1304x; 1.1304x over previous
"""Trainium2 Bass kernel for nn_Attention_69861938037658.

Computation per batch b (B=4096, S=200, H=128):
    proj  = X_b @ W1.T + (l_b @ W2.T)          # [S,H]
    hid   = tanh(proj)
    sc    = hid @ W3_w.T                       # [S]
    sc    = where(mask, -1e9, sc)
    attn  = softmax(sc)
    out_b = attn @ X_b                         # [H]

Sharding: pure data parallel, 512 batches per core on 8 cores.

Per-core structure (BB=64 batches per block, 8 blocks):
  - X loaded as [s, b, h]; per 2-batch group, PE-transpose to X^T columns,
    W1 matmul, tanh(+W2 l bias) on ACT.
  - Scores for all 32 groups of a block accumulate into one [32, 400] PSUM
    tile via "w3sel" stationaries ([128, 32] with w3 in column g, zeros
    elsewhere) - no per-row scatter DMAs.
  - Blockwide masked softmax on the [32, 400] two-segment layout.
  - Weighted sum via per-batch stationary-X matmuls into spaced PSUM cols.
"""

import sys
import numpy as np

if "/opt/trn_rl_repo" not in sys.path:
    sys.path.insert(0, "/opt/trn_rl_repo")

B, S, H = 4096, 200, 128
NCORES = 8
BC = B // NCORES          # 512 batches per core
BB = 64                   # batches per block
NBLK = BC // BB           # 8 blocks
NEG = -1.0e9

_NC_CACHE = {}
_RUN_CACHE = {}


def _build(nblk=NBLK, bb=BB, reps=1):
    import concourse.bacc as bacc
    import concourse.tile as tile
    from concourse import mybir
    from concourse.masks import make_identity
    from contextlib import ExitStack

    f32 = mybir.dt.float32
    f32r = mybir.dt.float32r
    u8 = mybir.dt.uint8
    Tanh = mybir.ActivationFunctionType.Tanh
    Exp = mybir.ActivationFunctionType.Exp

    bcp = nblk * bb           # batches this core processes
    gb = bb // 2              # 2-batch groups per block

    nc = bacc.Bacc("TRN2", target_bir_lowering=False, debug=False)

    x = nc.dram_tensor("x", [bcp, S, H], f32, kind="ExternalInput")
    l = nc.dram_tensor("l", [bcp, H], f32, kind="ExternalInput")
    m = nc.dram_tensor("m", [bcp, S], u8, kind="ExternalInput")
    w1 = nc.dram_tensor("w1", [H, H], f32, kind="ExternalInput")
    w2 = nc.dram_tensor("w2", [H, H], f32, kind="ExternalInput")
    w3 = nc.dram_tensor("w3", [1, H], f32, kind="ExternalInput")
    out = nc.dram_tensor("out", [bcp, H], f32, kind="ExternalOutput")

    with tile.TileContext(nc) as tc, ExitStack() as ctx:
        singles = ctx.enter_context(tc.tile_pool(name="singles", bufs=1))
        xa_p = ctx.enter_context(tc.tile_pool(name="xa", bufs=2))
        xb_p = ctx.enter_context(tc.tile_pool(name="xb", bufs=2))
        xt_p = ctx.enter_context(tc.tile_pool(name="xt", bufs=4))
        hid_p = ctx.enter_context(tc.tile_pool(name="hid", bufs=4))
        sm_p = ctx.enter_context(tc.tile_pool(name="sm", bufs=2))
        small_p = ctx.enter_context(tc.tile_pool(name="small", bufs=3))
        o_p = ctx.enter_context(tc.tile_pool(name="o", bufs=2))
        xtps_p = ctx.enter_context(tc.tile_pool(name="xtps", bufs=2, space="PSUM"))
        pjps_p = ctx.enter_context(tc.tile_pool(name="pjps", bufs=2, space="PSUM"))
        scps_p = ctx.enter_context(tc.tile_pool(name="scps", bufs=2, space="PSUM"))
        fob_p = ctx.enter_context(tc.tile_pool(name="fob", bufs=2, space="PSUM"))

        # ---- constants / weights ----
        ident = singles.tile([128, 128], f32)
        make_identity(nc, ident)
        negt = singles.tile([gb, 2 * S], f32)
        nc.vector.memset(negt, NEG)

        w1sb = singles.tile([H, H], f32)
        w2sb = singles.tile([H, H], f32)
        w3sb = singles.tile([1, H], f32)
        nc.sync.dma_start(out=w1sb, in_=w1[:, :])
        nc.sync.dma_start(out=w2sb, in_=w2[:, :])
        nc.sync.dma_start(out=w3sb, in_=w3[:, :])

        identr = singles.tile([128, 128], f32r)
        nc.vector.tensor_copy(identr, ident)
        wfob = fob_p.tile([128, 512], f32, tag="fob")
        w1T = singles.tile([H, H], f32r)
        nc.tensor.transpose(wfob[:, 0:H], w1sb, ident)
        nc.vector.tensor_copy(w1T, wfob[:, 0:H])
        wfob2 = fob_p.tile([128, 512], f32, tag="fob")
        w2T = singles.tile([H, H], f32r)
        nc.tensor.transpose(wfob2[:, 0:H], w2sb, ident)
        nc.vector.tensor_copy(w2T, wfob2[:, 0:H])

        # w3sel[:, g, :] is a [H, gb] stationary: w3 in column g, else 0.
        nc.tensor.transpose(wfob2[:, 256:257], w3sb, ident[0:1, 0:1])
        w3self = singles.tile([H, gb, gb], f32)
        nc.vector.memset(w3self, 0.0)
        for g in range(gb):
            nc.vector.tensor_copy(w3self[:, g, g : g + 1], wfob2[:, 256:257])
        w3sel = singles.tile([H, gb, gb], f32r)
        nc.vector.tensor_copy(w3sel, w3self)

        for rep in range(reps):
            for blk in range(nblk):
                b0 = blk * bb

                # one psum scratch tile per block:
                #   cols 0:128   fo   (weighted-sum accum, spacing 2)
                #   cols 128:256 atps (attn transposes, 4 x gb)
                #   cols 256:320 ltps; 320:448 plnat (onps reuses 320:448
                #   at block end)
                fob = fob_p.tile([128, 512], f32, tag="fob")
                fo = fob[:, 0 : 2 * bb]
                atps = fob[:, 2 * bb : 2 * bb + 4 * gb].rearrange(
                    "p (a g) -> p a g", a=4)

                # ---- proj_last for this block: plt[h, b] = W2 @ L_blk^T ----
                lsb = small_p.tile([bb, H], f32, tag="lsb")
                nc.sync.dma_start(out=lsb, in_=l[b0 : b0 + bb, :])
                nc.tensor.transpose(fob[:, 256 : 256 + bb], lsb, ident[0:bb, 0:bb])
                lt = small_p.tile([H, bb], f32r, tag="lt")
                nc.vector.tensor_copy(lt, fob[:, 256 : 256 + bb])
                nc.tensor.matmul(fob[:, 320 : 320 + bb], w2T, lt, start=True, stop=True)
                plt = small_p.tile([H, bb], f32r, tag="plt")
                nc.vector.tensor_copy(plt, fob[:, 320 : 320 + bb])

                # mask in [g, (pair s)] layout: row g = batches 2g, 2g+1
                mskt = small_p.tile([gb, 2 * S], u8, tag="msk")
                nc.sync.dma_start(
                    out=mskt,
                    in_=m[b0 : b0 + bb, :].rearrange("(g t) s -> g (t s)", t=2),
                )

                # ---- X loads: [s, b, h], split across HWDGE queues ----
                xa = xa_p.tile([128, bb, H], f32)
                xb = xb_p.tile([72, bb, H], f32)
                qeng = [nc.sync, nc.scalar, nc.sync, nc.scalar]
                bq = bb // 4
                for c in range(4):
                    c0 = c * bq
                    qeng[c].dma_start(
                        out=xa[:, c0 : c0 + bq, :],
                        in_=x[b0 + c0 : b0 + c0 + bq, 0:128, :].rearrange(
                            "b s h -> s b h"))
                    qeng[(c + 1) % 4].dma_start(
                        out=xb[:, c0 : c0 + bq, :],
                        in_=x[b0 + c0 : b0 + c0 + bq, 128:200, :].rearrange(
                            "b s h -> s b h"))

                # ---- per 2-batch group: transpose -> proj -> tanh -> scores
                # software-pipelined: w1 lags transposes by 1 group, w3sel
                # lags by 2, so PE never waits on DVE copies / ACT tanh.
                scps = scps_p.tile([gb, 2 * S], f32)
                xts = {}
                hids = {}

                def stage_transpose(g):
                    i0, i1 = 2 * g, 2 * g + 1
                    xtps = xtps_p.tile([128, 400], f32)
                    nc.tensor.transpose(xtps[:, 0:128], xa[:, i0, :], ident)
                    nc.tensor.transpose(xtps[:, 128:200], xb[:, i0, :], ident[0:72, 0:72])
                    nc.tensor.transpose(xtps[:, 200:328], xa[:, i1, :], ident)
                    nc.tensor.transpose(xtps[:, 328:400], xb[:, i1, :], ident[0:72, 0:72])
                    xt = xt_p.tile([128, 400], f32r)
                    nc.vector.tensor_copy(xt, xtps)
                    xts[g] = xt

                def stage_proj(g):
                    i0 = 2 * g
                    pjps = pjps_p.tile([128, 400], f32)
                    nc.tensor.matmul(pjps, w1T, xts.pop(g), start=True, stop=False)
                    # + l @ W2^T per batch: identity stationary, plt columns
                    # broadcast along s as the moving operand
                    pl_bcast = plt[:, i0 : i0 + 2][:, :, None].broadcast_to([H, 2, S])
                    nc.tensor.matmul(pjps.rearrange("p (t s) -> p t s", t=2),
                                     identr, pl_bcast, start=False, stop=True)
                    hid = hid_p.tile([128, 400], f32r)
                    nc.scalar.activation(hid, pjps, Tanh)
                    hids[g] = hid

                def stage_score(g):
                    nc.tensor.matmul(scps, w3sel[:, g, :], hids.pop(g),
                                     start=(g == 0), stop=(g == gb - 1),
                                     skip_group_check=True)

                for g in range(gb + 2):
                    if g < gb:
                        stage_transpose(g)
                    if 1 <= g < gb + 1:
                        stage_proj(g - 1)
                    if g >= 2:
                        stage_score(g - 2)

                # ---- masked softmax over S, rows = 2-batch groups ----
                # scores are bounded by ||w3||_1 (~10), so exp without
                # max-subtraction is safe in f32; masked entries underflow
                # to exactly 0.
                sc = sm_p.tile([gb, 2 * S], f32, tag="sc")
                nc.vector.tensor_copy(sc, scps)
                nc.vector.copy_predicated(sc, mskt, negt)
                pb = sm_p.tile([gb, 2 * S], f32, tag="pb")
                zE = small_p.tile([gb, 1], f32, tag="zE")
                zO = small_p.tile([gb, 1], f32, tag="zO")
                nc.scalar.activation(pb, sc, Exp)
                nc.vector.tensor_reduce(zE, pb[:, 0:200], mybir.AxisListType.X,
                                        mybir.AluOpType.add)
                nc.vector.tensor_reduce(zO, pb[:, 200:400], mybir.AxisListType.X,
                                        mybir.AluOpType.add)
                rzE = small_p.tile([gb, 1], f32, tag="rzE")
                rzO = small_p.tile([gb, 1], f32, tag="rzO")
                nc.vector.reciprocal(rzE, zE)
                nc.vector.reciprocal(rzO, zO)
                attn = sm_p.tile([gb, 2 * S], f32, tag="attn")
                nc.vector.tensor_scalar_mul(attn[:, 0:200], pb[:, 0:200], rzE)
                nc.vector.tensor_scalar_mul(attn[:, 200:400], pb[:, 200:400], rzO)

                # ---- transpose attn into per-batch columns ----
                # segments: 0 = even batches s0:128, 1 = even s128:200,
                #           2 = odd  batches s0:128, 3 = odd  s128:200
                nc.tensor.transpose(atps[:, 0, :], attn[:, 0:128], ident[0:gb, 0:gb])
                nc.tensor.transpose(atps[0:72, 1, :], attn[:, 128:200], ident[0:gb, 0:gb])
                nc.tensor.transpose(atps[:, 2, :], attn[:, 200:328], ident[0:gb, 0:gb])
                nc.tensor.transpose(atps[0:72, 3, :], attn[:, 328:400], ident[0:gb, 0:gb])
                attT = small_p.tile([128, 4, gb], f32, tag="attT")
                nc.vector.tensor_copy(attT[:, 0:4:2, :], atps[:, 0:4:2, :])
                nc.vector.tensor_copy(attT[0:72, 1:4:2, :], atps[0:72, 1:4:2, :])

                # ---- weighted sum: fo[h, 2*i] = sum_s attn[s,i] * X[s,i,h]
                # (spacing 2 keeps each accumulation column on its own 8B
                # PSUM cacheline)
                for i in range(bb):
                    g, odd = divmod(i, 2)
                    ca = attT[:, 2 * odd, g : g + 1]
                    cb = attT[0:72, 2 * odd + 1, g : g + 1]
                    nc.tensor.matmul(fo[:, 2 * i : 2 * i + 1], xa[:, i, :], ca,
                                     start=True, stop=False)
                    nc.tensor.matmul(fo[:, 2 * i : 2 * i + 1], xb[:, i, :], cb,
                                     start=False, stop=True)

                outT = o_p.tile([128, bb], f32, tag="outT")
                nc.vector.tensor_copy(outT, fo[:, 0 : 2 * bb : 2])
                nc.tensor.transpose(fob[0:bb, 320:448], outT, ident)
                onat = o_p.tile([bb, H], f32, tag="onat")
                nc.vector.tensor_copy(onat, fob[0:bb, 320:448])
                nc.sync.dma_start(out=out[b0 : b0 + bb, :], in_=onat)

    nc.finalize()
    return nc


def _get_nc(nblk=NBLK, bb=BB, reps=1):
    key = (nblk, bb, reps)
    if key not in _NC_CACHE:
        _NC_CACHE[key] = _build(nblk, bb, reps)
    return _NC_CACHE[key]


def _get_runner(nc, n_cores=NCORES):
    """Compile nc into a cached sharded-jit callable over n_cores devices.

    Returns (fn, in_names, out_names) where fn takes GLOBAL input arrays
    (axis 0 = n_cores * per-core) in in_names order and returns global
    output arrays.
    """
    key = id(nc)
    if key in _RUN_CACHE:
        return _RUN_CACHE[key]
    import jax
    from jax.sharding import Mesh, PartitionSpec
    try:
        from jax.experimental.shard_map import shard_map
    except Exception:
        from jax.shard_map import shard_map
    from concourse import mybir
    from concourse import bass2jax

    bass2jax.install_neuronx_cc_hook()

    partition_name = (nc.partition_id_tensor.name
                      if nc.partition_id_tensor else None)
    in_names, out_names, out_avals, zero_shapes = [], [], [], []
    for alloc in nc.m.functions[0].allocations:
        if not isinstance(alloc, mybir.MemoryLocationSet):
            continue
        name = alloc.memorylocations[0].name
        if alloc.kind == "ExternalInput":
            if name != partition_name:
                in_names.append(name)
        elif alloc.kind == "ExternalOutput":
            out_names.append(name)
            shape = tuple(alloc.tensor_shape)
            dtype = mybir.dt.np(alloc.dtype)
            out_avals.append(jax.core.ShapedArray(shape, dtype))
            zero_shapes.append((shape, dtype))
    n_params = len(in_names)
    n_outs = len(out_names)
    all_names = in_names + out_names
    if partition_name is not None:
        all_names = all_names + [partition_name]
    donate = tuple(range(n_params, n_params + n_outs))

    def _body(*args):
        operands = list(args)
        if partition_name is not None:
            operands.append(bass2jax.partition_id_tensor())
        outs = bass2jax._bass_exec_p.bind(
            *operands,
            out_avals=tuple(out_avals),
            in_names=tuple(all_names),
            out_names=tuple(out_names),
            lowering_input_output_aliases=(),
            sim_require_finite=True,
            sim_require_nnan=True,
            nc=nc,
        )
        return tuple(outs)

    devices = jax.devices()[:n_cores]
    assert len(devices) == n_cores, f"need {n_cores} devices, have {len(jax.devices())}"
    mesh = Mesh(np.asarray(devices), ("core",))
    in_specs = (PartitionSpec("core"),) * (n_params + n_outs)
    out_specs = (PartitionSpec("core"),) * n_outs
    fn = jax.jit(
        shard_map(_body, mesh=mesh, in_specs=in_specs, out_specs=out_specs,
                  check_rep=False),
        donate_argnums=donate,
        keep_unused=True,
    )
    entry = (fn, in_names, out_names, zero_shapes)
    _RUN_CACHE[key] = entry
    return entry


def _global_inputs(all_memory, last_memory, mask, W1, W2, W3_w):
    """Name -> global array (axis 0 shards across cores with zero copies
    for the big tensors)."""
    return {
        "x": np.ascontiguousarray(all_memory, dtype=np.float32),
        "l": np.ascontiguousarray(last_memory.reshape(B, H), dtype=np.float32),
        "m": np.ascontiguousarray(mask).view(np.uint8),
        "w1": np.tile(np.ascontiguousarray(W1, dtype=np.float32), (NCORES, 1)),
        "w2": np.tile(np.ascontiguousarray(W2, dtype=np.float32), (NCORES, 1)),
        "w3": np.tile(np.ascontiguousarray(W3_w, dtype=np.float32), (NCORES, 1)),
    }


def run_global(in_map, nc=None, reps=1):
    """Run the kernel on global inputs; returns dict of global outputs."""
    if nc is None:
        nc = _get_nc(reps=reps)
    fn, in_names, out_names, zero_shapes = _get_runner(nc)
    args = [in_map[name] for name in in_names]
    zeros = [np.zeros((NCORES * s[0], *s[1:]), d) for (s, d) in zero_shapes]
    outs = fn(*args, *zeros)
    return {name: np.asarray(outs[i]) for i, name in enumerate(out_names)}


def kernel(all_memory, last_memory, mask, W1, W2, W3_w, W3_b):
    # W3_b shifts every score equally; softmax is shift-invariant, so it
    # cancels (and it is zeros in setup_inputs).
    in_map = _global_inputs(all_memory, last_memory, mask, W1, W2, W3_w)
    outs = run_global(in_map)
    return outs["out"].astype(np.float32)


# revision 20
# speedup vs baseline: 1.2226x; 1.0816x over previous
"""Trainium2 Bass kernel for nn_Attention_69861938037658.

Computation per batch b (B=4096, S=200, H=128):
    proj  = X_b @ W1.T + (l_b @ W2.T)          # [S,H]
    hid   = tanh(proj)
    sc    = hid @ W3_w.T                       # [S]
    sc    = where(mask, -1e9, sc)
    attn  = softmax(sc)
    out_b = attn @ X_b                         # [H]

Sharding: pure data parallel, 512 batches per core on 8 cores.

Per-core structure (BB=64 batches per block, 8 blocks):
  - X loaded as [s, b, h]; per 2-batch group, PE-transpose to X^T columns,
    W1 matmul, tanh(+W2 l bias) on ACT.
  - Scores for all 32 groups of a block accumulate into one [32, 400] PSUM
    tile via "w3sel" stationaries ([128, 32] with w3 in column g, zeros
    elsewhere) - no per-row scatter DMAs.
  - Blockwide masked softmax on the [32, 400] two-segment layout.
  - Weighted sum via per-batch stationary-X matmuls into spaced PSUM cols.
"""

import sys
import numpy as np

if "/opt/trn_rl_repo" not in sys.path:
    sys.path.insert(0, "/opt/trn_rl_repo")

B, S, H = 4096, 200, 128
NCORES = 8
BC = B // NCORES          # 512 batches per core
BB = 64                   # batches per block
NBLK = BC // BB           # 8 blocks
NEG = -1.0e9

_NC_CACHE = {}
_RUN_CACHE = {}


def _build(nblk=NBLK, bb=BB, reps=1):
    import concourse.bacc as bacc
    import concourse.tile as tile
    from concourse import mybir
    from concourse.masks import make_identity
    from contextlib import ExitStack

    f32 = mybir.dt.float32
    f32r = mybir.dt.float32r
    u8 = mybir.dt.uint8
    Tanh = mybir.ActivationFunctionType.Tanh
    Exp = mybir.ActivationFunctionType.Exp

    bcp = nblk * bb           # batches this core processes
    gb = bb // 2              # 2-batch groups per block

    nc = bacc.Bacc("TRN2", target_bir_lowering=False, debug=False)

    x = nc.dram_tensor("x", [bcp, S, H], f32, kind="ExternalInput")
    l = nc.dram_tensor("l", [bcp, H], f32, kind="ExternalInput")
    m = nc.dram_tensor("m", [bcp, S], u8, kind="ExternalInput")
    w1 = nc.dram_tensor("w1", [H, H], f32, kind="ExternalInput")
    w2 = nc.dram_tensor("w2", [H, H], f32, kind="ExternalInput")
    w3 = nc.dram_tensor("w3", [1, H], f32, kind="ExternalInput")
    out = nc.dram_tensor("out", [bcp, H], f32, kind="ExternalOutput")

    with tile.TileContext(nc) as tc, ExitStack() as ctx:
        singles = ctx.enter_context(tc.tile_pool(name="singles", bufs=1))
        xa_p = ctx.enter_context(tc.tile_pool(name="xa", bufs=2))
        xb_p = ctx.enter_context(tc.tile_pool(name="xb", bufs=2))
        xt_p = ctx.enter_context(tc.tile_pool(name="xt", bufs=4))
        hid_p = ctx.enter_context(tc.tile_pool(name="hid", bufs=4))
        sm_p = ctx.enter_context(tc.tile_pool(name="sm", bufs=2))
        small_p = ctx.enter_context(tc.tile_pool(name="small", bufs=3))
        o_p = ctx.enter_context(tc.tile_pool(name="o", bufs=2))
        xtps_p = ctx.enter_context(tc.tile_pool(name="xtps", bufs=2, space="PSUM"))
        pjps_p = ctx.enter_context(tc.tile_pool(name="pjps", bufs=2, space="PSUM"))
        scps_p = ctx.enter_context(tc.tile_pool(name="scps", bufs=2, space="PSUM"))
        fob_p = ctx.enter_context(tc.tile_pool(name="fob", bufs=2, space="PSUM"))

        # ---- constants / weights ----
        ident = singles.tile([128, 128], f32)
        make_identity(nc, ident)
        negt = singles.tile([gb, 2 * S], f32)
        nc.vector.memset(negt, NEG)

        w1sb = singles.tile([H, H], f32)
        w2sb = singles.tile([H, H], f32)
        w3sb = singles.tile([1, H], f32)
        nc.sync.dma_start(out=w1sb, in_=w1[:, :])
        nc.sync.dma_start(out=w2sb, in_=w2[:, :])
        nc.sync.dma_start(out=w3sb, in_=w3[:, :])

        identr = singles.tile([128, 128], f32r)
        nc.vector.tensor_copy(identr, ident)
        wfob = fob_p.tile([128, 512], f32, tag="fob")
        w1T = singles.tile([H, H], f32r)
        nc.tensor.transpose(wfob[:, 0:H], w1sb, ident)
        nc.vector.tensor_copy(w1T, wfob[:, 0:H])
        wfob2 = fob_p.tile([128, 512], f32, tag="fob")
        w2T = singles.tile([H, H], f32r)
        nc.tensor.transpose(wfob2[:, 0:H], w2sb, ident)
        nc.vector.tensor_copy(w2T, wfob2[:, 0:H])

        # w3sel[:, g, :] is a [H, gb] stationary: w3 in column g, else 0.
        nc.tensor.transpose(wfob2[:, 256:257], w3sb, ident[0:1, 0:1])
        w3self = singles.tile([H, gb, gb], f32)
        nc.vector.memset(w3self, 0.0)
        for g in range(gb):
            nc.vector.tensor_copy(w3self[:, g, g : g + 1], wfob2[:, 256:257])
        w3sel = singles.tile([H, gb, gb], f32r)
        nc.vector.tensor_copy(w3sel, w3self)

        for rep in range(reps):
            for blk in range(nblk):
                b0 = blk * bb

                # one psum scratch tile per block:
                #   cols 0:128   fo   (weighted-sum accum, spacing 2)
                #   cols 128:256 atps (attn transposes, 4 x gb)
                #   cols 256:320 ltps; 320:448 plnat (onps reuses 320:448
                #   at block end)
                fob = fob_p.tile([128, 512], f32, tag="fob")
                fo = fob[:, 0 : 2 * bb]
                atps = fob[:, 2 * bb : 2 * bb + 4 * gb].rearrange(
                    "p (a g) -> p a g", a=4)

                # ---- proj_last for this block: plt[h, b] = W2 @ L_blk^T ----
                lsb = small_p.tile([bb, H], f32, tag="lsb")
                nc.sync.dma_start(out=lsb, in_=l[b0 : b0 + bb, :])
                nc.tensor.transpose(fob[:, 256 : 256 + bb], lsb, ident[0:bb, 0:bb])
                lt = small_p.tile([H, bb], f32r, tag="lt")
                nc.vector.tensor_copy(lt, fob[:, 256 : 256 + bb])
                nc.tensor.matmul(fob[:, 320 : 320 + bb], w2T, lt, start=True, stop=True)
                plt = small_p.tile([H, bb], f32r, tag="plt")
                nc.vector.tensor_copy(plt, fob[:, 320 : 320 + bb])

                # mask in [g, (pair s)] layout: row g = batches 2g, 2g+1
                mskt = small_p.tile([gb, 2 * S], u8, tag="msk")
                nc.sync.dma_start(
                    out=mskt,
                    in_=m[b0 : b0 + bb, :].rearrange("(g t) s -> g (t s)", t=2),
                )

                # ---- X loads: [s, b, h] ----
                xa = xa_p.tile([128, bb, H], f32)
                xb = xb_p.tile([72, bb, H], f32)
                nc.sync.dma_start(
                    out=xa, in_=x[b0 : b0 + bb, 0:128, :].rearrange("b s h -> s b h"))
                nc.sync.dma_start(
                    out=xb, in_=x[b0 : b0 + bb, 128:200, :].rearrange("b s h -> s b h"))

                # ---- per 2-batch group: transpose -> proj -> tanh -> scores
                # software-pipelined: w1 lags transposes by 1 group, w3sel
                # lags by 2, so PE never waits on DVE copies / ACT tanh.
                scps = scps_p.tile([gb, 2 * S], f32)
                xts = {}
                hids = {}

                def stage_transpose(g):
                    i0, i1 = 2 * g, 2 * g + 1
                    xtps = xtps_p.tile([128, 400], f32)
                    nc.tensor.transpose(xtps[:, 0:128], xa[:, i0, :], ident)
                    nc.tensor.transpose(xtps[:, 128:200], xb[:, i0, :], ident[0:72, 0:72])
                    nc.tensor.transpose(xtps[:, 200:328], xa[:, i1, :], ident)
                    nc.tensor.transpose(xtps[:, 328:400], xb[:, i1, :], ident[0:72, 0:72])
                    xt = xt_p.tile([128, 400], f32r)
                    nc.vector.tensor_copy(xt, xtps)
                    xts[g] = xt

                def stage_proj(g):
                    i0 = 2 * g
                    pjps = pjps_p.tile([128, 400], f32)
                    nc.tensor.matmul(pjps, w1T, xts.pop(g), start=True, stop=False)
                    # + l @ W2^T per batch: identity stationary, plt columns
                    # broadcast along s as the moving operand
                    pl_bcast = plt[:, i0 : i0 + 2][:, :, None].broadcast_to([H, 2, S])
                    nc.tensor.matmul(pjps.rearrange("p (t s) -> p t s", t=2),
                                     identr, pl_bcast, start=False, stop=True)
                    hid = hid_p.tile([128, 400], f32r)
                    nc.scalar.activation(hid, pjps, Tanh)
                    hids[g] = hid

                def stage_score(g):
                    nc.tensor.matmul(scps, w3sel[:, g, :], hids.pop(g),
                                     start=(g == 0), stop=(g == gb - 1),
                                     skip_group_check=True)

                for g in range(gb + 2):
                    if g < gb:
                        stage_transpose(g)
                    if 1 <= g < gb + 1:
                        stage_proj(g - 1)
                    if g >= 2:
                        stage_score(g - 2)

                # ---- masked softmax over S, rows = 2-batch groups ----
                # scores are bounded by ||w3||_1 (~10), so exp without
                # max-subtraction is safe in f32; masked entries underflow
                # to exactly 0.
                sc = sm_p.tile([gb, 2 * S], f32, tag="sc")
                nc.vector.tensor_copy(sc, scps)
                nc.vector.copy_predicated(sc, mskt, negt)
                pb = sm_p.tile([gb, 2 * S], f32, tag="pb")
                zE = small_p.tile([gb, 1], f32, tag="zE")
                zO = small_p.tile([gb, 1], f32, tag="zO")
                nc.scalar.activation(pb, sc, Exp)
                nc.vector.tensor_reduce(zE, pb[:, 0:200], mybir.AxisListType.X,
                                        mybir.AluOpType.add)
                nc.vector.tensor_reduce(zO, pb[:, 200:400], mybir.AxisListType.X,
                                        mybir.AluOpType.add)
                rzE = small_p.tile([gb, 1], f32, tag="rzE")
                rzO = small_p.tile([gb, 1], f32, tag="rzO")
                nc.vector.reciprocal(rzE, zE)
                nc.vector.reciprocal(rzO, zO)
                attn = sm_p.tile([gb, 2 * S], f32, tag="attn")
                nc.vector.tensor_scalar_mul(attn[:, 0:200], pb[:, 0:200], rzE)
                nc.vector.tensor_scalar_mul(attn[:, 200:400], pb[:, 200:400], rzO)

                # ---- transpose attn into per-batch columns ----
                # segments: 0 = even batches s0:128, 1 = even s128:200,
                #           2 = odd  batches s0:128, 3 = odd  s128:200
                nc.tensor.transpose(atps[:, 0, :], attn[:, 0:128], ident[0:gb, 0:gb])
                nc.tensor.transpose(atps[0:72, 1, :], attn[:, 128:200], ident[0:gb, 0:gb])
                nc.tensor.transpose(atps[:, 2, :], attn[:, 200:328], ident[0:gb, 0:gb])
                nc.tensor.transpose(atps[0:72, 3, :], attn[:, 328:400], ident[0:gb, 0:gb])
                attT = small_p.tile([128, 4, gb], f32, tag="attT")
                nc.vector.tensor_copy(attT[:, 0:4:2, :], atps[:, 0:4:2, :])
                nc.vector.tensor_copy(attT[0:72, 1:4:2, :], atps[0:72, 1:4:2, :])

                # ---- weighted sum: fo[h, 2*i] = sum_s attn[s,i] * X[s,i,h]
                # (spacing 2 keeps each accumulation column on its own 8B
                # PSUM cacheline)
                for i in range(bb):
                    g, odd = divmod(i, 2)
                    ca = attT[:, 2 * odd, g : g + 1]
                    cb = attT[0:72, 2 * odd + 1, g : g + 1]
                    nc.tensor.matmul(fo[:, 2 * i : 2 * i + 1], xa[:, i, :], ca,
                                     start=True, stop=False)
                    nc.tensor.matmul(fo[:, 2 * i : 2 * i + 1], xb[:, i, :], cb,
                                     start=False, stop=True)

                outT = o_p.tile([128, bb], f32, tag="outT")
                nc.vector.tensor_copy(outT, fo[:, 0 : 2 * bb : 2])
                nc.tensor.transpose(fob[0:bb, 320:448], outT, ident)
                onat = o_p.tile([bb, H], f32, tag="onat")
                nc.vector.tensor_copy(onat, fob[0:bb, 320:448])
                nc.sync.dma_start(out=out[b0 : b0 + bb, :], in_=onat)

    nc.finalize()
    return nc


def _get_nc(nblk=NBLK, bb=BB, reps=1):
    key = (nblk, bb, reps)
    if key not in _NC_CACHE:
        _NC_CACHE[key] = _build(nblk, bb, reps)
    return _NC_CACHE[key]


def _get_runner(nc, n_cores=NCORES):
    """Compile nc into a cached sharded-jit callable over n_cores devices.

    Returns (fn, in_names, out_names) where fn takes GLOBAL input arrays
    (axis 0 = n_cores * per-core) in in_names order and returns global
    output arrays.
    """
    key = id(nc)
    if key in _RUN_CACHE:
        return _RUN_CACHE[key]
    import jax
    from jax.sharding import Mesh, PartitionSpec
    try:
        from jax.experimental.shard_map import shard_map
    except Exception:
        from jax.shard_map import shard_map
    from concourse import mybir
    from concourse import bass2jax

    bass2jax.install_neuronx_cc_hook()

    partition_name = (nc.partition_id_tensor.name
                      if nc.partition_id_tensor else None)
    in_names, out_names, out_avals, zero_shapes = [], [], [], []
    for alloc in nc.m.functions[0].allocations:
        if not isinstance(alloc, mybir.MemoryLocationSet):
            continue
        name = alloc.memorylocations[0].name
        if alloc.kind == "ExternalInput":
            if name != partition_name:
                in_names.append(name)
        elif alloc.kind == "ExternalOutput":
            out_names.append(name)
            shape = tuple(alloc.tensor_shape)
            dtype = mybir.dt.np(alloc.dtype)
            out_avals.append(jax.core.ShapedArray(shape, dtype))
            zero_shapes.append((shape, dtype))
    n_params = len(in_names)
    n_outs = len(out_names)
    all_names = in_names + out_names
    if partition_name is not None:
        all_names = all_names + [partition_name]
    donate = tuple(range(n_params, n_params + n_outs))

    def _body(*args):
        operands = list(args)
        if partition_name is not None:
            operands.append(bass2jax.partition_id_tensor())
        outs = bass2jax._bass_exec_p.bind(
            *operands,
            out_avals=tuple(out_avals),
            in_names=tuple(all_names),
            out_names=tuple(out_names),
            lowering_input_output_aliases=(),
            sim_require_finite=True,
            sim_require_nnan=True,
            nc=nc,
        )
        return tuple(outs)

    devices = jax.devices()[:n_cores]
    assert len(devices) == n_cores, f"need {n_cores} devices, have {len(jax.devices())}"
    mesh = Mesh(np.asarray(devices), ("core",))
    in_specs = (PartitionSpec("core"),) * (n_params + n_outs)
    out_specs = (PartitionSpec("core"),) * n_outs
    fn = jax.jit(
        shard_map(_body, mesh=mesh, in_specs=in_specs, out_specs=out_specs,
                  check_rep=False),
        donate_argnums=donate,
        keep_unused=True,
    )
    entry = (fn, in_names, out_names, zero_shapes)
    _RUN_CACHE[key] = entry
    return entry


def _global_inputs(all_memory, last_memory, mask, W1, W2, W3_w):
    """Name -> global array (axis 0 shards across cores with zero copies
    for the big tensors)."""
    return {
        "x": np.ascontiguousarray(all_memory, dtype=np.float32),
        "l": np.ascontiguousarray(last_memory.reshape(B, H), dtype=np.float32),
        "m": np.ascontiguousarray(mask).view(np.uint8),
        "w1": np.tile(np.ascontiguousarray(W1, dtype=np.float32), (NCORES, 1)),
        "w2": np.tile(np.ascontiguousarray(W2, dtype=np.float32), (NCORES, 1)),
        "w3": np.tile(np.ascontiguousarray(W3_w, dtype=np.float32), (NCORES, 1)),
    }


def run_global(in_map, nc=None, reps=1):
    """Run the kernel on global inputs; returns dict of global outputs."""
    if nc is None:
        nc = _get_nc(reps=reps)
    fn, in_names, out_names, zero_shapes = _get_runner(nc)
    args = [in_map[name] for name in in_names]
    zeros = [np.zeros((NCORES * s[0], *s[1:]), d) for (s, d) in zero_shapes]
    outs = fn(*args, *zeros)
    return {name: np.asarray(outs[i]) for i, name in enumerate(out_names)}


def kernel(all_memory, last_memory, mask, W1, W2, W3_w, W3_b):
    # W3_b shifts every score equally; softmax is shift-invariant, so it
    # cancels (and it is zeros in setup_inputs).
    in_map = _global_inputs(all_memory, last_memory, mask, W1, W2, W3_w)
    outs = run_global(in_map)
    return outs["out"].astype(np.float32)
